# revision 5
# baseline (speedup 1.0000x reference)
"""Trainium2 Bass kernel for a WaveNet-style gated residual block.

Reference computation (per batch b):
    mel_c = cond_w @ mel + cond_b                  # [2R, T]
    wav_d = causal dilated conv(wav, dil_w) + dil_b  # [2R, T], kernel 2, dilation 8
    x     = tanh(mel_c[:R] + wav_d[:R]) * sigmoid(mel_c[R:] + wav_d[R:])
    skip  = skip_w @ x + skip_b                    # [S, T]
    resid = res_w @ x + res_b + wav                # [R, T]

Sharding: data-parallel over batch B=16 across 8 NeuronCores (2 batches per
core); weights replicated.  All convs are 1x1 (matmuls over the channel dim)
except the dilated conv, which is two shifted 1x1 matmuls accumulated in PSUM.
"""

import numpy as np

import concourse.bacc as bacc
import concourse.tile as tile
from concourse import mybir
from concourse.bass_utils import run_bass_kernel_spmd

# Problem shapes (hardcoded; must match the grader's inputs)
B, T = 16, 8192
N_MELS, DIL, R, S_OUT = 80, 8, 128, 240
N_CORES = 8
B_LOC = B // N_CORES  # 2 batches per core

CH = 2048  # time chunk per DMA round
NSUB = 4   # 512-wide compute subtiles per chunk
SUB = CH // NSUB  # 512 (= max fp32 matmul free dim / one PSUM bank)

F32 = mybir.dt.float32

_CACHE = {}


def _build_nc():
    nc = bacc.Bacc("TRN2", target_bir_lowering=False)

    mel = nc.dram_tensor("mel", [B_LOC, N_MELS, T], F32, kind="ExternalInput")
    wav = nc.dram_tensor("wav", [B_LOC, R, T], F32, kind="ExternalInput")
    # Weights, pre-transposed to [K, M] (stationary lhsT layout).
    w_cond = nc.dram_tensor("w_cond", [N_MELS, 2 * R], F32, kind="ExternalInput")
    w_dil0 = nc.dram_tensor("w_dil0", [128, 2 * R], F32, kind="ExternalInput")
    w_dil1 = nc.dram_tensor("w_dil1", [128, 2 * R], F32, kind="ExternalInput")
    w_skip = nc.dram_tensor("w_skip", [128, S_OUT], F32, kind="ExternalInput")
    w_res = nc.dram_tensor("w_res", [128, R], F32, kind="ExternalInput")
    ident = nc.dram_tensor("ident", [128, 128], F32, kind="ExternalInput")
    # bias columns: 0 = (cond_b+dil_b)[:128], 1 = (cond_b+dil_b)[128:],
    #               2 = skip_b[:128], 3 = skip_b[128:240] (padded), 4 = res_b
    biases = nc.dram_tensor("biases", [128, 5], F32, kind="ExternalInput")

    skip = nc.dram_tensor("skip", [B_LOC, S_OUT, T], F32, kind="ExternalOutput")
    resid = nc.dram_tensor("resid", [B_LOC, R, T], F32, kind="ExternalOutput")

    TANH = mybir.ActivationFunctionType.Tanh
    SIGM = mybir.ActivationFunctionType.Sigmoid

    with tile.TileContext(nc) as tc:
        with (
            tc.tile_pool(name="consts", bufs=1) as consts,
            tc.tile_pool(name="inp", bufs=2) as inp,
            tc.tile_pool(name="xp", bufs=2) as xp,
            tc.tile_pool(name="gate", bufs=3) as gate,
            tc.tile_pool(name="outp", bufs=2) as outp,
            tc.tile_pool(name="zps", bufs=2, space="PSUM") as zps,
            tc.tile_pool(name="ops", bufs=1, space="PSUM") as ops,
        ):
            w_cond_sb = consts.tile([N_MELS, 2 * R], F32)
            w_dil0_sb = consts.tile([128, 2 * R], F32)
            w_dil1_sb = consts.tile([128, 2 * R], F32)
            w_skip_sb = consts.tile([128, S_OUT], F32)
            w_res_sb = consts.tile([128, R], F32)
            ident_sb = consts.tile([128, 128], F32)
            bias_sb = consts.tile([128, 5], F32)
            nc.sync.dma_start(w_cond_sb[:], w_cond[:])
            nc.sync.dma_start(w_dil0_sb[:], w_dil0[:])
            nc.sync.dma_start(w_dil1_sb[:], w_dil1[:])
            nc.sync.dma_start(w_skip_sb[:], w_skip[:])
            nc.sync.dma_start(w_res_sb[:], w_res[:])
            nc.sync.dma_start(ident_sb[:], ident[:])
            nc.sync.dma_start(bias_sb[:], biases[:])

            pending = None  # (x_sb, wav_sb, s, chunk_bufs, b, c)

            def emit_out(st):
                x_sb, wav_sb, s, (skip0_sb, skip1_sb, res_sb), b, c = st
                sl = slice(s * SUB, (s + 1) * SUB)
                ps0 = ops.tile([128, SUB], F32, tag="ps0")
                ps1 = ops.tile([112, SUB], F32, tag="ps1")
                pr = ops.tile([128, SUB], F32, tag="pr")
                nc.tensor.matmul(ps0, w_skip_sb[:, 0:128], x_sb[:, sl],
                                 start=True, stop=True)
                nc.tensor.matmul(ps1, w_skip_sb[:, 128:S_OUT], x_sb[:, sl],
                                 start=True, stop=True)
                nc.tensor.matmul(pr, w_res_sb[:], x_sb[:, sl],
                                 start=True, stop=False)
                # resid += wav via identity matmul (keeps the add off DVE/ACT)
                nc.tensor.matmul(pr, ident_sb[:],
                                 wav_sb[:, DIL + s * SUB: DIL + (s + 1) * SUB],
                                 start=False, stop=True)
                nc.vector.tensor_scalar_add(skip0_sb[:, sl], ps0, bias_sb[:, 2:3])
                nc.vector.tensor_scalar_add(skip1_sb[:, sl], ps1, bias_sb[:112, 3:4])
                nc.scalar.add(res_sb[:, sl], pr, bias_sb[:, 4:5])
                if s == NSUB - 1:
                    csl = slice(c * CH, (c + 1) * CH)
                    nc.sync.dma_start(skip[b, 0:128, csl], skip0_sb[:])
                    nc.sync.dma_start(skip[b, 128:S_OUT, csl], skip1_sb[:])
                    nc.sync.dma_start(resid[b, :, csl], res_sb[:])

            for b in range(B_LOC):
                for c in range(T // CH):
                    csl = slice(c * CH, (c + 1) * CH)
                    mel_sb = inp.tile([N_MELS, CH], F32)
                    wav_sb = inp.tile([128, CH + DIL], F32)
                    nc.sync.dma_start(mel_sb[:], mel[b, :, csl])
                    if c == 0:
                        nc.vector.memset(wav_sb[:, 0:DIL], 0.0)
                        nc.sync.dma_start(wav_sb[:, DIL:], wav[b, :, csl])
                    else:
                        nc.sync.dma_start(
                            wav_sb[:], wav[b, :, c * CH - DIL:(c + 1) * CH])

                    x_sb = xp.tile([128, CH], F32)
                    skip0_sb = outp.tile([128, CH], F32)
                    skip1_sb = outp.tile([112, CH], F32)
                    res_sb = outp.tile([128, CH], F32)
                    chunk_bufs = (skip0_sb, skip1_sb, res_sb)

                    for s in range(NSUB):
                        sl = slice(s * SUB, (s + 1) * SUB)
                        # z = cond_w@mel + dil_w0@wav[t-8] + dil_w1@wav[t]
                        zt = zps.tile([128, SUB], F32, tag="zt")
                        zb = zps.tile([128, SUB], F32, tag="zb")
                        w0 = wav_sb[:, s * SUB: s * SUB + SUB]          # t-8 tap
                        w1 = wav_sb[:, DIL + s * SUB: DIL + s * SUB + SUB]  # t tap
                        nc.tensor.matmul(zt, w_cond_sb[:, 0:128], mel_sb[:, sl],
                                         start=True, stop=False)
                        nc.tensor.matmul(zt, w_dil0_sb[:, 0:128], w0,
                                         start=False, stop=False)
                        nc.tensor.matmul(zt, w_dil1_sb[:, 0:128], w1,
                                         start=False, stop=True)
                        nc.tensor.matmul(zb, w_cond_sb[:, 128:256], mel_sb[:, sl],
                                         start=True, stop=False)
                        nc.tensor.matmul(zb, w_dil0_sb[:, 128:256], w0,
                                         start=False, stop=False)
                        nc.tensor.matmul(zb, w_dil1_sb[:, 128:256], w1,
                                         start=False, stop=True)

                        xt = gate.tile([128, SUB], F32, tag="xt")
                        xs = gate.tile([128, SUB], F32, tag="xs")
                        nc.scalar.activation(xt, zt, TANH, bias=bias_sb[:, 0:1])
                        nc.scalar.activation(xs, zb, SIGM, bias=bias_sb[:, 1:2])
                        nc.vector.tensor_mul(x_sb[:, sl], xt, xs)

                        if pending is not None:
                            emit_out(pending)
                        pending = (x_sb, wav_sb, s, chunk_bufs, b, c)

            emit_out(pending)

    nc.compile()
    return nc


def _prep_static_inputs(cond_w, cond_b, dil_w, dil_b, skip_w, skip_b, res_w, res_b):
    f = np.float32
    w_cond = np.ascontiguousarray(cond_w[:, :, 0].T, f)
    w_dil0 = np.ascontiguousarray(dil_w[:, :, 0].T, f)
    w_dil1 = np.ascontiguousarray(dil_w[:, :, 1].T, f)
    w_skip = np.ascontiguousarray(skip_w[:, :, 0].T, f)
    w_res = np.ascontiguousarray(res_w[:, :, 0].T, f)
    ident = np.eye(128, dtype=f)
    zb = (cond_b + dil_b).astype(f)
    biases = np.zeros((128, 5), f)
    biases[:, 0] = zb[:128]
    biases[:, 1] = zb[128:]
    biases[:, 2] = skip_b[:128]
    biases[:112, 3] = skip_b[128:]
    biases[:, 4] = res_b
    return {
        "w_cond": w_cond, "w_dil0": w_dil0, "w_dil1": w_dil1,
        "w_skip": w_skip, "w_res": w_res, "ident": ident, "biases": biases,
    }


def _run(inputs, trace=False):
    if "nc" not in _CACHE:
        _CACHE["nc"] = _build_nc()
    nc = _CACHE["nc"]

    static = _prep_static_inputs(
        np.asarray(inputs["cond_w"], np.float32),
        np.asarray(inputs["cond_b"], np.float32),
        np.asarray(inputs["dil_w"], np.float32),
        np.asarray(inputs["dil_b"], np.float32),
        np.asarray(inputs["skip_w"], np.float32),
        np.asarray(inputs["skip_b"], np.float32),
        np.asarray(inputs["res_w"], np.float32),
        np.asarray(inputs["res_b"], np.float32),
    )
    mel = np.asarray(inputs["mel"], np.float32)
    wav = np.asarray(inputs["wav"], np.float32)

    in_maps = []
    for i in range(N_CORES):
        m = dict(static)
        m["mel"] = np.ascontiguousarray(mel[i * B_LOC:(i + 1) * B_LOC])
        m["wav"] = np.ascontiguousarray(wav[i * B_LOC:(i + 1) * B_LOC])
        in_maps.append(m)

    res = run_bass_kernel_spmd(nc, in_maps, core_ids=list(range(N_CORES)),
                               trace=trace)
    skip = np.concatenate([r["skip"] for r in res.results], axis=0)
    resid = np.concatenate([r["resid"] for r in res.results], axis=0)
    return (skip, resid), res


def kernel(**inputs):
    return _run(inputs)[0]


# revision 15
# speedup vs baseline: 1.0369x; 1.0369x over previous
"""Trainium2 Bass kernel for a WaveNet-style gated residual block.

Reference computation (per batch b):
    mel_c = cond_w @ mel + cond_b                  # [2R, T]
    wav_d = causal dilated conv(wav, dil_w) + dil_b  # [2R, T], kernel 2, dilation 8
    x     = tanh(mel_c[:R] + wav_d[:R]) * sigmoid(mel_c[R:] + wav_d[R:])
    skip  = skip_w @ x + skip_b                    # [S, T]
    resid = res_w @ x + res_b + wav                # [R, T]

Sharding: data-parallel over batch B=16 across 8 NeuronCores (2 batches per
core); weights replicated.  All convs are 1x1 (matmuls over the channel dim)
except the dilated conv, which is two shifted 1x1 matmuls accumulated in PSUM.
"""

import numpy as np

import concourse.bacc as bacc
import concourse.tile as tile
from concourse import mybir
from concourse.bass_utils import run_bass_kernel_spmd

# Problem shapes (hardcoded; must match the grader's inputs)
B, T = 16, 8192
N_MELS, DIL, R, S_OUT = 80, 8, 128, 240
N_CORES = 8
B_LOC = B // N_CORES  # 2 batches per core

CH = 2048  # time chunk per DMA round
NSUB = 4   # 512-wide compute subtiles per chunk
SUB = CH // NSUB  # 512 (= max fp32 matmul free dim / one PSUM bank)

F32 = mybir.dt.float32
# fp32r streams through the PE at 1 cycle/row (vs 4 for exact fp32) at the
# cost of a ~12-bit multiplicand mantissa; accumulate is still fp32 in PSUM.
# Matmul operands must already be rounded to fp32r, so the host pre-rounds
# mel/wav/weights (round-half-even at mantissa bit 12, matching the HW cast)
# and the DRAM tensors are declared float32r.
F32R = mybir.dt.float32r


def _round_fp32r(a):
    b = np.ascontiguousarray(a, np.float32).view(np.uint32).astype(np.uint64)
    r = (b + 0x7FF + ((b >> 12) & 1)) & 0xFFFFF000
    return r.astype(np.uint32).view(np.float32)

_CACHE = {}


def _build_nc():
    nc = bacc.Bacc("TRN2", target_bir_lowering=False)

    mel = nc.dram_tensor("mel", [B_LOC, N_MELS, T], F32R, kind="ExternalInput")
    wav = nc.dram_tensor("wav", [B_LOC, R, T], F32R, kind="ExternalInput")
    # Weights, pre-transposed to [K, M] (stationary lhsT layout).
    w_cond = nc.dram_tensor("w_cond", [N_MELS, 2 * R], F32R, kind="ExternalInput")
    w_dil0 = nc.dram_tensor("w_dil0", [128, 2 * R], F32R, kind="ExternalInput")
    w_dil1 = nc.dram_tensor("w_dil1", [128, 2 * R], F32R, kind="ExternalInput")
    w_skip = nc.dram_tensor("w_skip", [128, S_OUT], F32R, kind="ExternalInput")
    w_res = nc.dram_tensor("w_res", [128, R], F32R, kind="ExternalInput")
    ident = nc.dram_tensor("ident", [128, 128], F32R, kind="ExternalInput")
    # bias columns: 0 = (cond_b+dil_b)[:128], 1 = (cond_b+dil_b)[128:],
    #               2 = skip_b[:128], 3 = skip_b[128:240] (padded), 4 = res_b
    biases = nc.dram_tensor("biases", [128, 5], F32, kind="ExternalInput")

    skip = nc.dram_tensor("skip", [B_LOC, S_OUT, T], F32, kind="ExternalOutput")
    resid = nc.dram_tensor("resid", [B_LOC, R, T], F32, kind="ExternalOutput")

    TANH = mybir.ActivationFunctionType.Tanh
    SIGM = mybir.ActivationFunctionType.Sigmoid

    with tile.TileContext(nc) as tc:
        with (
            tc.tile_pool(name="consts", bufs=1) as consts,
            tc.tile_pool(name="inp", bufs=2) as inp,
            tc.tile_pool(name="xp", bufs=2) as xp,
            tc.tile_pool(name="gate", bufs=3) as gate,
            tc.tile_pool(name="outp", bufs=2) as outp,
            tc.tile_pool(name="zps", bufs=2, space="PSUM") as zps,
            tc.tile_pool(name="ops", bufs=1, space="PSUM") as ops,
        ):
            w_cond_sb = consts.tile([N_MELS, 2 * R], F32R)
            w_dil0_sb = consts.tile([128, 2 * R], F32R)
            w_dil1_sb = consts.tile([128, 2 * R], F32R)
            w_skip_sb = consts.tile([128, S_OUT], F32R)
            w_res_sb = consts.tile([128, R], F32R)
            ident_sb = consts.tile([128, 128], F32R)
            bias_sb = consts.tile([128, 5], F32)
            nc.sync.dma_start(w_cond_sb[:], w_cond[:])
            nc.sync.dma_start(w_dil0_sb[:], w_dil0[:])
            nc.sync.dma_start(w_dil1_sb[:], w_dil1[:])
            nc.sync.dma_start(w_skip_sb[:], w_skip[:])
            nc.sync.dma_start(w_res_sb[:], w_res[:])
            nc.sync.dma_start(ident_sb[:], ident[:])
            nc.sync.dma_start(bias_sb[:], biases[:])

            pending = None  # (x_sb, wav_sb, s, chunk_bufs, b, c)

            def emit_out(st):
                x_sb, wav_sb, s, (skip0_sb, skip1_sb, res_sb), b, c = st
                sl = slice(s * SUB, (s + 1) * SUB)
                ps0 = ops.tile([128, SUB], F32, tag="ps0")
                ps1 = ops.tile([112, SUB], F32, tag="ps1")
                pr = ops.tile([128, SUB], F32, tag="pr")
                nc.tensor.matmul(ps0, w_skip_sb[:, 0:128], x_sb[:, sl],
                                 start=True, stop=True)
                nc.tensor.matmul(ps1, w_skip_sb[:, 128:S_OUT], x_sb[:, sl],
                                 start=True, stop=True)
                nc.tensor.matmul(pr, w_res_sb[:], x_sb[:, sl],
                                 start=True, stop=False)
                # resid += wav via identity matmul (keeps the add off DVE/ACT)
                nc.tensor.matmul(pr, ident_sb[:],
                                 wav_sb[:, DIL + s * SUB: DIL + (s + 1) * SUB],
                                 start=False, stop=True)
                nc.vector.tensor_scalar_add(skip0_sb[:, sl], ps0, bias_sb[:, 2:3])
                nc.vector.tensor_scalar_add(skip1_sb[:, sl], ps1, bias_sb[:112, 3:4])
                nc.scalar.add(res_sb[:, sl], pr, bias_sb[:, 4:5])
                if s == NSUB - 1:
                    csl = slice(c * CH, (c + 1) * CH)
                    nc.sync.dma_start(skip[b, 0:128, csl], skip0_sb[:])
                    nc.sync.dma_start(skip[b, 128:S_OUT, csl], skip1_sb[:])
                    nc.sync.dma_start(resid[b, :, csl], res_sb[:])

            for b in range(B_LOC):
                for c in range(T // CH):
                    csl = slice(c * CH, (c + 1) * CH)
                    mel_sb = inp.tile([N_MELS, CH], F32R)
                    wav_sb = inp.tile([128, CH + DIL], F32R)
                    nc.sync.dma_start(mel_sb[:], mel[b, :, csl])
                    if c == 0:
                        # cols 0:DIL stay garbage; the first subtile's t-8 tap
                        # skips output cols 0:DIL (causally zero) instead.
                        nc.sync.dma_start(wav_sb[:, DIL:], wav[b, :, csl])
                    else:
                        nc.sync.dma_start(
                            wav_sb[:], wav[b, :, c * CH - DIL:(c + 1) * CH])

                    x_sb = xp.tile([128, CH], F32R)
                    skip0_sb = outp.tile([128, CH], F32)
                    skip1_sb = outp.tile([112, CH], F32)
                    res_sb = outp.tile([128, CH], F32)
                    chunk_bufs = (skip0_sb, skip1_sb, res_sb)

                    for s in range(NSUB):
                        sl = slice(s * SUB, (s + 1) * SUB)
                        # z = cond_w@mel + dil_w0@wav[t-8] + dil_w1@wav[t]
                        zt = zps.tile([128, SUB], F32, tag="zt")
                        zb = zps.tile([128, SUB], F32, tag="zb")
                        # first DIL outputs of the whole batch have no t-8 tap
                        off = DIL if (c == 0 and s == 0) else 0
                        w0 = wav_sb[:, s * SUB + off: s * SUB + SUB]    # t-8 tap
                        w1 = wav_sb[:, DIL + s * SUB: DIL + s * SUB + SUB]  # t tap
                        nc.tensor.matmul(zt, w_cond_sb[:, 0:128], mel_sb[:, sl],
                                         start=True, stop=False)
                        nc.tensor.matmul(zt[:, off:SUB], w_dil0_sb[:, 0:128], w0,
                                         start=False, stop=False)
                        nc.tensor.matmul(zt, w_dil1_sb[:, 0:128], w1,
                                         start=False, stop=True)
                        nc.tensor.matmul(zb, w_cond_sb[:, 128:256], mel_sb[:, sl],
                                         start=True, stop=False)
                        nc.tensor.matmul(zb[:, off:SUB], w_dil0_sb[:, 128:256], w0,
                                         start=False, stop=False)
                        nc.tensor.matmul(zb, w_dil1_sb[:, 128:256], w1,
                                         start=False, stop=True)

                        xt = gate.tile([128, SUB], F32, tag="xt")
                        xs = gate.tile([128, SUB], F32, tag="xs")
                        nc.scalar.activation(xt, zt, TANH, bias=bias_sb[:, 0:1])
                        nc.scalar.activation(xs, zb, SIGM, bias=bias_sb[:, 1:2])
                        nc.vector.tensor_mul(x_sb[:, sl], xt, xs)

                        if pending is not None:
                            emit_out(pending)
                        pending = (x_sb, wav_sb, s, chunk_bufs, b, c)

            emit_out(pending)

    nc.compile()
    return nc


def _prep_static_inputs(cond_w, cond_b, dil_w, dil_b, skip_w, skip_b, res_w, res_b):
    f = np.float32
    w_cond = _round_fp32r(cond_w[:, :, 0].T)
    w_dil0 = _round_fp32r(dil_w[:, :, 0].T)
    w_dil1 = _round_fp32r(dil_w[:, :, 1].T)
    w_skip = _round_fp32r(skip_w[:, :, 0].T)
    w_res = _round_fp32r(res_w[:, :, 0].T)
    ident = np.eye(128, dtype=f)
    zb = (cond_b + dil_b).astype(f)
    biases = np.zeros((128, 5), f)
    biases[:, 0] = zb[:128]
    biases[:, 1] = zb[128:]
    biases[:, 2] = skip_b[:128]
    biases[:112, 3] = skip_b[128:]
    biases[:, 4] = res_b
    return {
        "w_cond": w_cond, "w_dil0": w_dil0, "w_dil1": w_dil1,
        "w_skip": w_skip, "w_res": w_res, "ident": ident, "biases": biases,
    }


def _run(inputs, trace=False):
    if "nc" not in _CACHE:
        _CACHE["nc"] = _build_nc()
    nc = _CACHE["nc"]

    static = _prep_static_inputs(
        np.asarray(inputs["cond_w"], np.float32),
        np.asarray(inputs["cond_b"], np.float32),
        np.asarray(inputs["dil_w"], np.float32),
        np.asarray(inputs["dil_b"], np.float32),
        np.asarray(inputs["skip_w"], np.float32),
        np.asarray(inputs["skip_b"], np.float32),
        np.asarray(inputs["res_w"], np.float32),
        np.asarray(inputs["res_b"], np.float32),
    )
    mel = _round_fp32r(np.asarray(inputs["mel"], np.float32))
    wav = _round_fp32r(np.asarray(inputs["wav"], np.float32))

    in_maps = []
    for i in range(N_CORES):
        m = dict(static)
        m["mel"] = np.ascontiguousarray(mel[i * B_LOC:(i + 1) * B_LOC])
        m["wav"] = np.ascontiguousarray(wav[i * B_LOC:(i + 1) * B_LOC])
        in_maps.append(m)

    res = run_bass_kernel_spmd(nc, in_maps, core_ids=list(range(N_CORES)),
                               trace=trace)
    skip = np.concatenate([r["skip"] for r in res.results], axis=0)
    resid = np.concatenate([r["resid"] for r in res.results], axis=0)
    return (skip, resid), res


def kernel(**inputs):
    return _run(inputs)[0]


# revision 20
# speedup vs baseline: 1.0721x; 1.0339x over previous
"""Trainium2 Bass kernel for a WaveNet-style gated residual block.

Reference computation (per batch b):
    mel_c = cond_w @ mel + cond_b                  # [2R, T]
    wav_d = causal dilated conv(wav, dil_w) + dil_b  # [2R, T], kernel 2, dilation 8
    x     = tanh(mel_c[:R] + wav_d[:R]) * sigmoid(mel_c[R:] + wav_d[R:])
    skip  = skip_w @ x + skip_b                    # [S, T]
    resid = res_w @ x + res_b + wav                # [R, T]

Sharding: data-parallel over batch B=16 across 8 NeuronCores (2 batches per
core); weights replicated.  All convs are 1x1 (matmuls over the channel dim)
except the dilated conv, which is two shifted 1x1 matmuls accumulated in PSUM.
"""

import numpy as np

import concourse.bacc as bacc
import concourse.tile as tile
from concourse import mybir
from concourse.bass_utils import run_bass_kernel_spmd

# Problem shapes (hardcoded; must match the grader's inputs)
B, T = 16, 8192
N_MELS, DIL, R, S_OUT = 80, 8, 128, 240
N_CORES = 8
B_LOC = B // N_CORES  # 2 batches per core

CH = 2048  # time chunk per DMA round
NSUB = 4   # 512-wide compute subtiles per chunk
SUB = CH // NSUB  # 512 (= max fp32 matmul free dim / one PSUM bank)
CH2 = CH // 2     # store granularity (half chunk)
WPACK_COLS = 2 * R * 3 + S_OUT + R + 128  # 1264

F32 = mybir.dt.float32
# fp32r streams through the PE at 1 cycle/row (vs 4 for exact fp32) at the
# cost of a ~12-bit multiplicand mantissa; accumulate is still fp32 in PSUM.
# Matmul operands must already be rounded to fp32r, so the host pre-rounds
# mel/wav/weights (round-half-even at mantissa bit 12, matching the HW cast)
# and the DRAM tensors are declared float32r.
F32R = mybir.dt.float32r


def _round_fp32r(a):
    b = np.ascontiguousarray(a, np.float32).view(np.uint32).astype(np.uint64)
    r = (b + 0x7FF + ((b >> 12) & 1)) & 0xFFFFF000
    return r.astype(np.uint32).view(np.float32)

_CACHE = {}


def _build_nc():
    nc = bacc.Bacc("TRN2", target_bir_lowering=False)

    mel = nc.dram_tensor("mel", [B_LOC, N_MELS, T], F32R, kind="ExternalInput")
    wav = nc.dram_tensor("wav", [B_LOC, R, T], F32R, kind="ExternalInput")
    # All matmul weights packed into one tensor (single DMA), pre-transposed to
    # [K, M] (stationary lhsT layout).  Column blocks:
    #   0:256 w_cond (rows 80: zero) | 256:512 w_dil0 | 512:768 w_dil1
    #   | 768:1008 w_skip | 1008:1136 w_res | 1136:1264 identity
    wpack = nc.dram_tensor("wpack", [128, WPACK_COLS], F32R, kind="ExternalInput")
    # bias columns: 0 = (cond_b+dil_b)[:128], 1 = (cond_b+dil_b)[128:],
    #               2 = skip_b[:128], 3 = skip_b[128:240] (padded), 4 = res_b
    biases = nc.dram_tensor("biases", [128, 5], F32, kind="ExternalInput")

    skip = nc.dram_tensor("skip", [B_LOC, S_OUT, T], F32, kind="ExternalOutput")
    resid = nc.dram_tensor("resid", [B_LOC, R, T], F32, kind="ExternalOutput")

    TANH = mybir.ActivationFunctionType.Tanh
    SIGM = mybir.ActivationFunctionType.Sigmoid

    with tile.TileContext(nc) as tc:
        with (
            tc.tile_pool(name="consts", bufs=1) as consts,
            tc.tile_pool(name="inp", bufs=3) as inp,
            tc.tile_pool(name="xp", bufs=2) as xp,
            tc.tile_pool(name="gate", bufs=3) as gate,
            tc.tile_pool(name="outp", bufs=2) as outp,
            tc.tile_pool(name="zps", bufs=2, space="PSUM") as zps,
            tc.tile_pool(name="ops", bufs=1, space="PSUM") as ops,
        ):
            wpack_sb = consts.tile([128, WPACK_COLS], F32R)
            bias_sb = consts.tile([128, 5], F32)
            nc.sync.dma_start(wpack_sb[:], wpack[:])
            nc.sync.dma_start(bias_sb[:], biases[:])
            w_cond_sb = wpack_sb[:N_MELS, 0:256]
            w_dil0_sb = wpack_sb[:, 256:512]
            w_dil1_sb = wpack_sb[:, 512:768]
            w_skip_sb = wpack_sb[:, 768:1008]
            w_res_sb = wpack_sb[:, 1008:1136]
            ident_sb = wpack_sb[:, 1136:1264]

            pending = None  # (x_sb, wav_sb, s, chunk_bufs, b, c)

            def emit_out(st):
                x_sb, wav_sb, s, (skip0_sb, skip1_sb, res_sb), b, c = st
                sl = slice(s * SUB, (s + 1) * SUB)
                ps0 = ops.tile([128, SUB], F32, tag="ps0")
                ps1 = ops.tile([112, SUB], F32, tag="ps1")
                pr = ops.tile([128, SUB], F32, tag="pr")
                nc.tensor.matmul(ps0, w_skip_sb[:, 0:128], x_sb[:, sl],
                                 start=True, stop=True)
                nc.tensor.matmul(ps1, w_skip_sb[:, 128:S_OUT], x_sb[:, sl],
                                 start=True, stop=True)
                nc.tensor.matmul(pr, w_res_sb[:], x_sb[:, sl],
                                 start=True, stop=False)
                # resid += wav via identity matmul (keeps the add off DVE/ACT)
                nc.tensor.matmul(pr, ident_sb[:],
                                 wav_sb[:, DIL + s * SUB: DIL + (s + 1) * SUB],
                                 start=False, stop=True)
                nc.vector.tensor_scalar_add(skip0_sb[:, sl], ps0, bias_sb[:, 2:3])
                nc.vector.tensor_scalar_add(skip1_sb[:, sl], ps1, bias_sb[:112, 3:4])
                nc.scalar.add(res_sb[:, sl], pr, bias_sb[:, 4:5])
                if s % 2 == 1:
                    # store each completed half chunk via the (otherwise idle)
                    # gpsimd SWDGE queue so output drains overlap compute
                    h = s // 2
                    hsl = slice(h * CH2, (h + 1) * CH2)
                    dsl = slice(c * CH + h * CH2, c * CH + (h + 1) * CH2)
                    nc.gpsimd.dma_start(skip[b, 0:128, dsl], skip0_sb[:, hsl])
                    nc.gpsimd.dma_start(skip[b, 128:S_OUT, dsl], skip1_sb[:, hsl])
                    nc.gpsimd.dma_start(resid[b, :, dsl], res_sb[:, hsl])

            for b in range(B_LOC):
                for c in range(T // CH):
                    csl = slice(c * CH, (c + 1) * CH)
                    mel_sb = inp.tile([N_MELS, CH], F32R)
                    wav_sb = inp.tile([128, CH + DIL], F32R)
                    nc.sync.dma_start(mel_sb[:], mel[b, :, csl])
                    if c == 0:
                        # cols 0:DIL stay garbage; the first subtile's t-8 tap
                        # skips output cols 0:DIL (causally zero) instead.
                        nc.sync.dma_start(wav_sb[:, DIL:], wav[b, :, csl])
                    else:
                        nc.sync.dma_start(
                            wav_sb[:], wav[b, :, c * CH - DIL:(c + 1) * CH])

                    x_sb = xp.tile([128, CH], F32R)
                    skip0_sb = outp.tile([128, CH], F32)
                    skip1_sb = outp.tile([112, CH], F32)
                    res_sb = outp.tile([128, CH], F32)
                    chunk_bufs = (skip0_sb, skip1_sb, res_sb)

                    for s in range(NSUB):
                        sl = slice(s * SUB, (s + 1) * SUB)
                        # z = cond_w@mel + dil_w0@wav[t-8] + dil_w1@wav[t]
                        zt = zps.tile([128, SUB], F32, tag="zt")
                        zb = zps.tile([128, SUB], F32, tag="zb")
                        # first DIL outputs of the whole batch have no t-8 tap
                        off = DIL if (c == 0 and s == 0) else 0
                        w0 = wav_sb[:, s * SUB + off: s * SUB + SUB]    # t-8 tap
                        w1 = wav_sb[:, DIL + s * SUB: DIL + s * SUB + SUB]  # t tap
                        nc.tensor.matmul(zt, w_cond_sb[:, 0:128], mel_sb[:, sl],
                                         start=True, stop=False)
                        nc.tensor.matmul(zt[:, off:SUB], w_dil0_sb[:, 0:128], w0,
                                         start=False, stop=False)
                        nc.tensor.matmul(zt, w_dil1_sb[:, 0:128], w1,
                                         start=False, stop=True)
                        nc.tensor.matmul(zb, w_cond_sb[:, 128:256], mel_sb[:, sl],
                                         start=True, stop=False)
                        nc.tensor.matmul(zb[:, off:SUB], w_dil0_sb[:, 128:256], w0,
                                         start=False, stop=False)
                        nc.tensor.matmul(zb, w_dil1_sb[:, 128:256], w1,
                                         start=False, stop=True)

                        xt = gate.tile([128, SUB], F32, tag="xt")
                        xs = gate.tile([128, SUB], F32, tag="xs")
                        nc.scalar.activation(xt, zt, TANH, bias=bias_sb[:, 0:1])
                        nc.scalar.activation(xs, zb, SIGM, bias=bias_sb[:, 1:2])
                        nc.vector.tensor_mul(x_sb[:, sl], xt, xs)

                        if pending is not None:
                            emit_out(pending)
                        pending = (x_sb, wav_sb, s, chunk_bufs, b, c)

            emit_out(pending)

    nc.compile()
    return nc


def _prep_static_inputs(cond_w, cond_b, dil_w, dil_b, skip_w, skip_b, res_w, res_b):
    f = np.float32
    wpack = np.zeros((128, WPACK_COLS), f)
    wpack[:N_MELS, 0:256] = _round_fp32r(cond_w[:, :, 0].T)
    wpack[:, 256:512] = _round_fp32r(dil_w[:, :, 0].T)
    wpack[:, 512:768] = _round_fp32r(dil_w[:, :, 1].T)
    wpack[:, 768:1008] = _round_fp32r(skip_w[:, :, 0].T)
    wpack[:, 1008:1136] = _round_fp32r(res_w[:, :, 0].T)
    wpack[:, 1136:1264] = np.eye(128, dtype=f)
    zb = (cond_b + dil_b).astype(f)
    biases = np.zeros((128, 5), f)
    biases[:, 0] = zb[:128]
    biases[:, 1] = zb[128:]
    biases[:, 2] = skip_b[:128]
    biases[:112, 3] = skip_b[128:]
    biases[:, 4] = res_b
    return {"wpack": wpack, "biases": biases}


def _run(inputs, trace=False):
    if "nc" not in _CACHE:
        _CACHE["nc"] = _build_nc()
    nc = _CACHE["nc"]

    static = _prep_static_inputs(
        np.asarray(inputs["cond_w"], np.float32),
        np.asarray(inputs["cond_b"], np.float32),
        np.asarray(inputs["dil_w"], np.float32),
        np.asarray(inputs["dil_b"], np.float32),
        np.asarray(inputs["skip_w"], np.float32),
        np.asarray(inputs["skip_b"], np.float32),
        np.asarray(inputs["res_w"], np.float32),
        np.asarray(inputs["res_b"], np.float32),
    )
    mel = _round_fp32r(np.asarray(inputs["mel"], np.float32))
    wav = _round_fp32r(np.asarray(inputs["wav"], np.float32))

    in_maps = []
    for i in range(N_CORES):
        m = dict(static)
        m["mel"] = np.ascontiguousarray(mel[i * B_LOC:(i + 1) * B_LOC])
        m["wav"] = np.ascontiguousarray(wav[i * B_LOC:(i + 1) * B_LOC])
        in_maps.append(m)

    res = run_bass_kernel_spmd(nc, in_maps, core_ids=list(range(N_CORES)),
                               trace=trace)
    skip = np.concatenate([r["skip"] for r in res.results], axis=0)
    resid = np.concatenate([r["resid"] for r in res.results], axis=0)
    return (skip, resid), res


def kernel(**inputs):
    return _run(inputs)[0]
